# revision 1
# baseline (speedup 1.0000x reference)
"""TRN2 Bass kernel for nn_LocalAttention (B=4, T=2048, C=1024, window=16).

Sharding: 8 cores = (batch b, row-half h). Each core computes K^T/V for its
whole batch (duplicated across the 2 cores of a batch) and attention +
projections for its own 1024 rows (two 512-row chunks; h=0 gets global
chunks {0,3}, h=1 gets {1,2}; slot 0 = denser chunk).

All matmuls run in fp32r (TF32-like, ~1.5e-4 rel err, 4x fp32 speed). Raw
fp32 bytes are declared as fp32r at the DRAM boundary - the PE rounds
internally (validated: identical error to explicit cast-DMA).

Orientation trick: host passes X^T and W^T so every matmul is natural:
  K^T = (Wk^T)^T @ X^T        [C, T]     (DRAM scratch)
  V   = (X^T)^T @ Wv^T        [T, C]     (DRAM scratch)
  Q^T = (Wq^T)^T @ X_own^T    [C, 1024]  (SBUF resident)
  S^T = (K^T_blk)^T @ Q^T_chunk  -> [keys, rows]; softmax-over-keys is a
        partition reduction done by a ones-vector matmul, and E^T feeds
  Y^T = V_blk^T @ E^T            [C, rows]
  Z^T = (Wo^T)^T @ Y^T           [C, rows]

Sparsity: mask keeps j >= i - 16 (reverse-causal), so each 512-row chunk's
kept key-block set is a SUFFIX {b..15}; processing key blocks in descending
order (position p -> block 15-p) makes every kept set a static PREFIX.
Chunk slot 0 runs 16 positions, slot 1 runs 9 - uniform across cores, the
data-driven is_ge mask zeroes over-included blocks. Mask applied
multiplicatively post-exp (scores are O(6), no overflow without max-sub).
"""
import numpy as np

import concourse.bass as bass
import concourse.mybir as mybir
import concourse.tile as tile
from concourse import bacc
from concourse import bass_utils

N_CORES = 8
B, T, C = 4, 2048, 1024
WINDOW = 16
TOWN = T // 2          # own rows per core
CHUNK = 512            # rows per processing chunk
NCHUNK = TOWN // CHUNK  # 2
CI = C // 128          # 8 contraction blocks
CO = C // 128          # 8 output blocks
KB = T // 128          # 16 key blocks
TCH = T // CHUNK       # 4 t-chunks in phase A
SLOT_KBS = (16, 9)     # key-block positions per chunk slot (descending order)
F32 = mybir.dt.float32
F32R = mybir.dt.float32r

_NC_CACHE = {}


def build():
    if "nc" in _NC_CACHE:
        return _NC_CACHE["nc"]
    nc = bacc.Bacc("TRN2", target_bir_lowering=False, debug=False,
                   num_devices=N_CORES)
    xt = nc.dram_tensor("xt", [C, T], F32R, kind="ExternalInput").ap()
    xtq = nc.dram_tensor("xtq", [C, TOWN], F32R, kind="ExternalInput").ap()
    wqt = nc.dram_tensor("wqt", [C, C], F32R, kind="ExternalInput").ap()
    wkt = nc.dram_tensor("wkt", [C, C], F32R, kind="ExternalInput").ap()
    wvt = nc.dram_tensor("wvt", [C, C], F32R, kind="ExternalInput").ap()
    wot = nc.dram_tensor("wot", [C, C], F32R, kind="ExternalInput").ap()
    keyidx16 = nc.dram_tensor("keyidx16", [128, KB], F32, kind="ExternalInput").ap()
    rowidxb = nc.dram_tensor("rowidxb", [128, TOWN], F32, kind="ExternalInput").ap()
    zt = nc.dram_tensor("zt", [C, TOWN], F32, kind="ExternalOutput").ap()

    xt3 = xt.rearrange("(ko ki) t -> ki ko t", ki=128)
    xtq3 = xtq.rearrange("(ko ki) t -> ki ko t", ki=128)
    w3 = {w.tensor.name: w.rearrange("(ko ki) c -> ki ko c", ki=128)
          for w in (wqt, wkt, wvt, wot)}

    inv_sqrt_c = float(1.0 / np.sqrt(C))

    with tile.TileContext(nc) as tc:
        with tc.tile_pool(name="res", bufs=1) as res, \
             tc.tile_pool(name="dram", bufs=1, space="DRAM") as dram:
            kt_d = dram.tile([128, CI, T], F32R)      # K^T  [ki, ko, t]
            v_d = dram.tile([128, KB, C], F32R)       # V    [ki, ko, c]
            qt_sb = res.tile([128, CI, TOWN], F32R, tag="qt")  # Q^T resident
            wo_sb = res.tile([128, CI, C], F32R, tag="wo")
            ki16_sb = res.tile([128, KB], F32, tag="ki16")
            nc.gpsimd.dma_start(ki16_sb[:], keyidx16[:])
            ones_row_f32 = res.tile([1, 128], F32, tag="onesrf")
            nc.vector.memset(ones_row_f32[:], 1.0)
            ones_1x128 = res.tile([1, 128], F32R, tag="o1")
            nc.vector.tensor_copy(ones_1x128[:], ones_row_f32[:])
            ones_col_f32 = res.tile([128, 1], F32, tag="onescf")
            nc.vector.memset(ones_col_f32[:], 1.0)
            ones_128x1 = res.tile([128, 1], F32R, tag="o2")
            nc.vector.tensor_copy(ones_128x1[:], ones_col_f32[:])

            # ================= Phase A: projections =========================
            with tc.tile_pool(name="wts", bufs=1) as wts, \
                 tc.tile_pool(name="xa", bufs=2) as xa, \
                 tc.tile_pool(name="stg", bufs=3) as stg, \
                 tc.tile_pool(name="ps_k", bufs=3, space="PSUM") as ps_k, \
                 tc.tile_pool(name="ps_v", bufs=2, space="PSUM") as ps_v, \
                 tc.tile_pool(name="ps_q", bufs=2, space="PSUM") as ps_q:
                wk_sb = wts.tile([128, CI, C], F32R, tag="wk")
                wv_sb = wts.tile([128, CI, C], F32R, tag="wv")
                wq_sb = wts.tile([128, CI, C], F32R, tag="wq")
                # first xt chunk before anything else on the sync queue
                xt_sbs = []
                xt_sb0 = xa.tile([128, CI, CHUNK], F32R, tag="xa")
                nc.sync.dma_start(xt_sb0[:], xt3[:, :, (TCH - 1) * CHUNK:TCH * CHUNK])
                for co in range(CO):  # per-column loads: co=0 unblocks MMs
                    nc.sync.dma_start(wk_sb[:, :, co * 128:(co + 1) * 128],
                                      w3["wkt"][:, :, co * 128:(co + 1) * 128])
                for ci in range(CI):
                    nc.scalar.dma_start(wv_sb[:, ci, :], w3["wvt"][:, ci, :])
                for ci in range(CI):
                    nc.scalar.dma_start(wq_sb[:, ci, :], w3["wqt"][:, ci, :])

                for tch in reversed(range(TCH)):
                    if tch == TCH - 1:
                        xt_sb = xt_sb0
                    else:
                        xt_sb = xa.tile([128, CI, CHUNK], F32R, tag="xa")
                        nc.sync.dma_start(
                            xt_sb[:], xt3[:, :, tch * CHUNK:(tch + 1) * CHUNK])
                    # K^T [cout, t]
                    for co in range(CO):
                        kps = ps_k.tile([128, CHUNK], F32, tag="kps")
                        for ci in range(CI):
                            nc.tensor.matmul(
                                kps[:], wk_sb[:, ci, co * 128:(co + 1) * 128],
                                xt_sb[:, ci, :], start=(ci == 0), stop=(ci == CI - 1))
                        kstage = stg.tile([128, CHUNK], F32R, tag="kstage")
                        nc.vector.tensor_copy(kstage[:], kps[:])
                        nc.sync.dma_start(
                            kt_d[:, co, tch * CHUNK:(tch + 1) * CHUNK], kstage[:])
                    # V [t, cout]
                    for tb in range(CHUNK // 128):
                        for half in range(2):
                            vps = ps_v.tile([128, 512], F32, tag="vps")
                            for ci in range(CI):
                                nc.tensor.matmul(
                                    vps[:], xt_sb[:, ci, tb * 128:(tb + 1) * 128],
                                    wv_sb[:, ci, half * 512:(half + 1) * 512],
                                    start=(ci == 0), stop=(ci == CI - 1))
                            vstage = stg.tile([128, 512], F32R, tag="vstage")
                            nc.vector.tensor_copy(vstage[:], vps[:])
                            nc.scalar.dma_start(
                                v_d[:, tch * (CHUNK // 128) + tb,
                                    half * 512:(half + 1) * 512], vstage[:])

                for qch in range(TOWN // CHUNK):
                    xq_sb = xa.tile([128, CI, CHUNK], F32R, tag="xa")
                    nc.sync.dma_start(
                        xq_sb[:], xtq3[:, :, qch * CHUNK:(qch + 1) * CHUNK])
                    for co in range(CO):
                        qps = ps_q.tile([128, CHUNK], F32, tag="qps")
                        for ci in range(CI):
                            nc.tensor.matmul(
                                qps[:], wq_sb[:, ci, co * 128:(co + 1) * 128],
                                xq_sb[:, ci, :], start=(ci == 0), stop=(ci == CI - 1))
                        nc.vector.tensor_copy(
                            qt_sb[:, co, qch * CHUNK:(qch + 1) * CHUNK], qps[:])

            # wo on the gpsimd (SWDGE) queue: latency-insensitive, keeps the
            # HW-DGE queues free for phase-B kt/v streams
            for ci in range(CI):
                nc.gpsimd.dma_start(wo_sb[:, ci, :], w3["wot"][:, ci, :])

            # ================= Phase B: attention + out-proj ================
            with tc.tile_pool(name="et", bufs=1) as etp, \
                 tc.tile_pool(name="ktb", bufs=4) as ktb_p, \
                 tc.tile_pool(name="vco", bufs=3) as vsp, \
                 tc.tile_pool(name="ysb", bufs=2) as ysb_p, \
                 tc.tile_pool(name="wb", bufs=2) as wb, \
                 tc.tile_pool(name="zst", bufs=3) as zstp, \
                 tc.tile_pool(name="ps_s", bufs=3, space="PSUM") as ps_s, \
                 tc.tile_pool(name="ps_sh", bufs=1, space="PSUM") as ps_sh, \
                 tc.tile_pool(name="ps_y", bufs=2, space="PSUM") as ps_y, \
                 tc.tile_pool(name="ps_z", bufs=2, space="PSUM") as ps_z:
                for ch in range(NCHUNK):
                    nkb = SLOT_KBS[ch]
                    rsl = slice(ch * CHUNK, (ch + 1) * CHUNK)
                    ri_b = wb.tile([128, CHUNK], F32, tag="rib")
                    nc.sync.dma_start(ri_b[:], rowidxb[:, rsl])

                    et = etp.tile([128, KB, CHUNK], F32R, tag="et")
                    # --- sweep 1a: scores + exp + mask (descending kb) ---
                    for p in range(nkb):
                        kb = KB - 1 - p
                        kt_b = ktb_p.tile([128, CI, 128], F32R, tag="ktb")
                        nc.scalar.dma_start(
                            kt_b[:], kt_d[:, :, kb * 128:(kb + 1) * 128])
                        sps = ps_s.tile([128, CHUNK], F32, tag="sps")
                        for ci in range(CI):
                            nc.tensor.matmul(
                                sps[:], kt_b[:, ci, :], qt_sb[:, ci, rsl],
                                start=(ci == 0), stop=(ci == CI - 1))
                        nc.scalar.activation(et[:, p, :], sps[:],
                                             mybir.ActivationFunctionType.Exp,
                                             scale=inv_sqrt_c)
                        mask = wb.tile([128, CHUNK], F32, tag="mask")
                        nc.vector.tensor_tensor(
                            mask[:], ki16_sb[:, kb:kb + 1].to_broadcast((128, CHUNK)),
                            ri_b[:], mybir.AluOpType.is_ge)
                        nc.vector.tensor_tensor(et[:, p, :], et[:, p, :], mask[:],
                                                mybir.AluOpType.mult)
                    # --- sweep 1b: key-sums via ones matmul ---
                    sums_ps = ps_sh.tile([1, CHUNK], F32, tag="shared")
                    for p in range(nkb):
                        nc.tensor.matmul(sums_ps[:], ones_128x1[:], et[:, p, :],
                                         start=(p == 0), stop=(p == nkb - 1))
                    recip = wb.tile([1, CHUNK], F32R, tag="recip")
                    with nc.allow_low_precision(reason="fp32r normalizer broadcast"):
                        nc.vector.reciprocal(recip[:], sums_ps[:])
                    rb_ps = ps_sh.tile([128, CHUNK], F32, tag="shared")
                    nc.tensor.matmul(rb_ps[:], ones_1x128[:], recip[:],
                                     start=True, stop=True)
                    rb_sb = wb.tile([128, CHUNK], F32, tag="rbsb")
                    nc.vector.tensor_copy(rb_sb[:], rb_ps[:])

                    # --- sweep 2: Y^T = V^T @ E^T per cout block ---
                    y_sb = ysb_p.tile([128, CO, CHUNK], F32R, tag="ysb")
                    for co in range(CO):
                        v_co = vsp.tile([128, KB, 128], F32R, tag="vco")
                        nc.sync.dma_start(
                            v_co[:, :nkb, :],
                            v_d[:, KB - nkb:, co * 128:(co + 1) * 128])
                        yps = ps_y.tile([128, CHUNK], F32, tag="yps")
                        for p in range(nkb):
                            nc.tensor.matmul(yps[:], v_co[:, nkb - 1 - p, :],
                                             et[:, p, :],
                                             start=(p == 0), stop=(p == nkb - 1))
                        nc.vector.tensor_copy(y_sb[:, co, :], yps[:])

                    # --- out-proj + normalize ---
                    for co in range(CO):
                        zps = ps_z.tile([128, CHUNK], F32, tag="zps")
                        for ci in range(CI):
                            nc.tensor.matmul(
                                zps[:], wo_sb[:, ci, co * 128:(co + 1) * 128],
                                y_sb[:, ci, :], start=(ci == 0), stop=(ci == CI - 1))
                        zst = zstp.tile([128, CHUNK], F32, tag="zst")
                        nc.vector.tensor_tensor(zst[:], zps[:], rb_sb[:],
                                                mybir.AluOpType.mult)
                        nc.sync.dma_start(zt[co * 128:(co + 1) * 128, rsl], zst[:])
    nc.compile()
    _NC_CACHE["nc"] = nc
    return nc


def make_in_maps(inputs):
    x = np.asarray(inputs["x"], dtype=np.float32)
    for bname in ("bq", "bk", "bv", "bo"):
        bval = np.asarray(inputs[bname])
        assert np.all(bval == 0.0), f"{bname} nonzero: unsupported fast path"
    wqt = np.ascontiguousarray(np.asarray(inputs["Wq"], np.float32).T)
    wkt = np.ascontiguousarray(np.asarray(inputs["Wk"], np.float32).T)
    wvt = np.ascontiguousarray(np.asarray(inputs["Wv"], np.float32).T)
    wot = np.ascontiguousarray(np.asarray(inputs["Wo"], np.float32).T)
    keyidx16 = (np.arange(T, dtype=np.float32).reshape(KB, 128).T + WINDOW
                ).copy()  # [128, KB]
    chunk_map = {0: (0, 3), 1: (1, 2)}  # slot 0 = denser chunk
    in_maps = []
    for core in range(N_CORES):
        b, h = divmod(core, 2)
        xt_b = np.ascontiguousarray(x[b].T)  # [C, T]
        ch0, ch1 = chunk_map[h]
        xtq = np.concatenate(
            [xt_b[:, ch0 * CHUNK:(ch0 + 1) * CHUNK],
             xt_b[:, ch1 * CHUNK:(ch1 + 1) * CHUNK]], axis=1)
        rowidx = np.concatenate(
            [np.arange(ch0 * CHUNK, (ch0 + 1) * CHUNK, dtype=np.float32),
             np.arange(ch1 * CHUNK, (ch1 + 1) * CHUNK, dtype=np.float32)])
        rowidxb = np.ascontiguousarray(
            np.broadcast_to(rowidx[None, :], (128, TOWN)))
        in_maps.append({
            "xt": xt_b, "xtq": np.ascontiguousarray(xtq),
            "wqt": wqt, "wkt": wkt, "wvt": wvt, "wot": wot,
            "keyidx16": keyidx16, "rowidxb": rowidxb,
        })
    return in_maps


def gather_output(results, dtype):
    out = np.empty((B, T, C), dtype=dtype)
    chunk_map = {0: (0, 3), 1: (1, 2)}
    for core in range(N_CORES):
        b, h = divmod(core, 2)
        y = results[core]["zt"].T  # [TOWN rows, C]
        ch0, ch1 = chunk_map[h]
        out[b, ch0 * CHUNK:(ch0 + 1) * CHUNK] = y[:CHUNK]
        out[b, ch1 * CHUNK:(ch1 + 1) * CHUNK] = y[CHUNK:]
    return out


def kernel(**inputs):
    nc = build()
    in_maps = make_in_maps(inputs)
    res = bass_utils.run_bass_kernel_spmd(nc, in_maps,
                                          core_ids=list(range(N_CORES)))
    return gather_output(res.results, np.asarray(inputs["x"]).dtype)



# revision 4
# speedup vs baseline: 1.2585x; 1.2585x over previous
"""TRN2 Bass kernel v2 for nn_LocalAttention (B=4, T=2048, C=1024, window=16).

Sharding: 8 cores = (batch b, half h). Each core handles 1024 rows of one
batch as four 256-row slots; h=0 gets global 256-chunks (0,3,4,7), h=1 gets
(1,2,5,6) (balanced by kept-key-block count, slot needs differ by <=2 from
the uniform caps (16,13,9,5)).

All matmul operands are bf16 (host-cast; PSUM accumulates fp32). K^T, V, Q^T
are SBUF-resident - no DRAM round-trip for attention operands.

Mask keeps j >= i - 16 (reverse-causal): each slot's kept key-block set is a
suffix; processing key blocks in DESCENDING order makes every kept set a
static prefix. The S sweep is kb-major: for each key block, one wide matmul
over all slots still needing it (slot spans are prefixes of the row axis
because slot caps are descending). E is packed [128 keys, sum(span)] bf16.
Only trailing span slots can touch the diagonal/boundary, so just those get
the data-driven is_ge mask (zeroes overflow positions too).

Optional USE_CC: each core projects K^T/V only for its half of T and a
pair-wise (cores 2b,2b+1) DRAM AllGather assembles the full K^T/V, halving
phase-A projection work. Staging reuses kt_sb/v_sb's first half; the gather
output overwrites both halves in global order on both cores.
"""
import numpy as np
import ml_dtypes

import concourse.bass as bass
import concourse.mybir as mybir
import concourse.tile as tile
from concourse import bacc
from concourse import bass_utils

N_CORES = 8
B, T, C = 4, 2048, 1024
WINDOW = 16
TOWN = T // 2           # own rows per core
RCH = 256               # rows per slot
NSLOT = TOWN // RCH     # 4
CI = C // 128           # 8 contraction blocks
CO = C // 128           # 8 output blocks
KB = T // 128           # 16 key blocks
CAPS = (16, 13, 9, 5)   # key-block positions per slot (descending kb)
CHUNK_MAP = {0: (0, 3, 4, 7), 1: (1, 2, 5, 6)}  # slot -> global 256-chunk

F32 = mybir.dt.float32
F32R = mybir.dt.float32r
BF16 = mybir.dt.bfloat16

USE_CC = False
CC_PAIRS = [[0, 1], [2, 3], [4, 5], [6, 7]]

# ---- static kb-major S-sweep structure ----
# span n(kb): number of slots (prefix of slot axis) needing key block kb
SPAN = {kb: sum(1 for cap in CAPS if cap >= KB - kb) for kb in range(KB)}
EOFF = {}
_off = 0
for _kb in range(KB - 1, -1, -1):
    EOFF[_kb] = _off
    _off += SPAN[_kb] * RCH
ETOT = _off  # 11008

# masked trailing slots per kb: slot s is mask-free at kb iff kb is strictly
# above the diagonal for BOTH cores' chunk at that slot
_MAXCHUNK = [max(CHUNK_MAP[0][s], CHUNK_MAP[1][s]) for s in range(NSLOT)]
NMASK = {}
for _kb in range(KB):
    n = SPAN[_kb]
    safe = [_kb >= 2 * _MAXCHUNK[s] + 2 for s in range(n)]
    # safe flags must be a prefix (maxchunk increases with s)
    assert safe == sorted(safe, reverse=True), (_kb, safe)
    NMASK[_kb] = n - sum(safe)

inv_sqrt_c = float(1.0 / np.sqrt(C))

_NC_CACHE = {}


def build():
    if "nc" in _NC_CACHE:
        return _NC_CACHE["nc"]
    nc = bacc.Bacc("TRN2", target_bir_lowering=False, debug=False,
                   num_devices=N_CORES)
    t_own = T // 2 if USE_CC else T
    xt = nc.dram_tensor("xt", [C, t_own], BF16, kind="ExternalInput").ap()
    xtq = nc.dram_tensor("xtq", [C, TOWN], BF16, kind="ExternalInput").ap()
    wqt = nc.dram_tensor("wqt", [C, C], BF16, kind="ExternalInput").ap()
    wkt = nc.dram_tensor("wkt", [C, C], BF16, kind="ExternalInput").ap()
    wvt = nc.dram_tensor("wvt", [C, C], BF16, kind="ExternalInput").ap()
    wot = nc.dram_tensor("wot", [C, C], BF16, kind="ExternalInput").ap()
    keyidx16 = nc.dram_tensor("keyidx16", [128, KB], F32, kind="ExternalInput").ap()
    rowidxb = nc.dram_tensor("rowidxb", [128, TOWN], F32, kind="ExternalInput").ap()
    zt = nc.dram_tensor("zt", [C, TOWN], F32, kind="ExternalOutput").ap()

    xt3 = xt.rearrange("(ko ki) t -> ki ko t", ki=128)
    xtq3 = xtq.rearrange("(ko ki) t -> ki ko t", ki=128)
    w3 = {w.tensor.name: w.rearrange("(ko ki) c -> ki ko c", ki=128)
          for w in (wqt, wkt, wvt, wot)}

    if USE_CC:
        cc_k_in = nc.dram_tensor("cc_k_in", [128, CI, T // 2], BF16).ap()
        cc_k_out = nc.dram_tensor("cc_k_out", [2, 128, CI, T // 2], BF16).ap()
        cc_v_in = nc.dram_tensor("cc_v_in", [128, KB // 2, C], BF16).ap()
        cc_v_out = nc.dram_tensor("cc_v_out", [2, 128, KB // 2, C], BF16).ap()

    with tile.TileContext(nc) as tc:
        with tc.tile_pool(name="res", bufs=1) as res:
            kt_sb = res.tile([128, CI, T], BF16, tag="kt")    # K^T resident
            v_sb = res.tile([128, KB, C], BF16, tag="v")      # V resident
            qt_sb = res.tile([128, CI, TOWN], BF16, tag="qt")  # Q^T resident
            wo_sb = res.tile([128, CI, C], BF16, tag="wo")
            ki16_sb = res.tile([128, KB], F32, tag="ki16")
            ri_sb = res.tile([128, TOWN], F32, tag="ri")
            nc.gpsimd.dma_start(ki16_sb[:], keyidx16[:])
            nc.gpsimd.dma_start(ri_sb[:], rowidxb[:])
            ones_col_f32 = res.tile([128, 1], F32, tag="onescf")
            nc.vector.memset(ones_col_f32[:], 1.0)
            ones_bf = res.tile([128, 1], BF16, tag="o1")
            nc.vector.tensor_copy(ones_bf[:], ones_col_f32[:])
            ones_row_f32 = res.tile([1, 128], F32, tag="onesrf")
            nc.vector.memset(ones_row_f32[:], 1.0)
            ones_1x128 = res.tile([1, 128], F32R, tag="o2")
            nc.vector.tensor_copy(ones_1x128[:], ones_row_f32[:])

            # ================= Phase A: projections =========================
            n_tch = t_own // 512
            with tc.tile_pool(name="wts", bufs=1) as wts, \
                 tc.tile_pool(name="xa", bufs=2) as xa, \
                 tc.tile_pool(name="ps_k", bufs=3, space="PSUM") as ps_k, \
                 tc.tile_pool(name="ps_v", bufs=2, space="PSUM") as ps_v, \
                 tc.tile_pool(name="ps_q", bufs=2, space="PSUM") as ps_q:
                wk_sb = wts.tile([128, CI, C], BF16, tag="wk")
                wv_sb = wts.tile([128, CI, C], BF16, tag="wv")
                wq_sb = wts.tile([128, CI, C], BF16, tag="wq")
                # first xt chunk before anything else on the sync queue
                xt_sb0 = xa.tile([128, CI, 512], BF16, tag="xa")
                nc.sync.dma_start(xt_sb0[:], xt3[:, :, (n_tch - 1) * 512:n_tch * 512])
                for co in range(CO):  # per-column loads: co=0 unblocks MMs
                    nc.sync.dma_start(wk_sb[:, :, co * 128:(co + 1) * 128],
                                      w3["wkt"][:, :, co * 128:(co + 1) * 128])
                for ci in range(CI):
                    nc.scalar.dma_start(wv_sb[:, ci, :], w3["wvt"][:, ci, :])
                for ci in range(CI):
                    nc.scalar.dma_start(wq_sb[:, ci, :], w3["wqt"][:, ci, :])

                for tch in reversed(range(n_tch)):
                    if tch == n_tch - 1:
                        xt_sb = xt_sb0
                    else:
                        xt_sb = xa.tile([128, CI, 512], BF16, tag="xa")
                        nc.sync.dma_start(
                            xt_sb[:], xt3[:, :, tch * 512:(tch + 1) * 512])
                    # K^T [cout, t] -> kt_sb
                    for co in range(CO):
                        kps = ps_k.tile([128, 512], F32, tag="kps")
                        for ci in range(CI):
                            nc.tensor.matmul(
                                kps[:], wk_sb[:, ci, co * 128:(co + 1) * 128],
                                xt_sb[:, ci, :], start=(ci == 0), stop=(ci == CI - 1))
                        nc.vector.tensor_copy(
                            kt_sb[:, co, tch * 512:(tch + 1) * 512], kps[:])
                    # V [t, cout] -> v_sb
                    for tb in range(4):
                        for half in range(2):
                            vps = ps_v.tile([128, 512], F32, tag="vps")
                            for ci in range(CI):
                                nc.tensor.matmul(
                                    vps[:], xt_sb[:, ci, tb * 128:(tb + 1) * 128],
                                    wv_sb[:, ci, half * 512:(half + 1) * 512],
                                    start=(ci == 0), stop=(ci == CI - 1))
                            nc.vector.tensor_copy(
                                v_sb[:, tch * 4 + tb,
                                     half * 512:(half + 1) * 512], vps[:])

                if USE_CC:
                    # stage own-half K^T/V (sitting in the first half of the
                    # resident tiles) out to DRAM, gather, and re-load full
                    for ci in range(CI):
                        nc.sync.dma_start(cc_k_in[:, ci, :],
                                          kt_sb[:, ci, 0:T // 2])
                    nc.gpsimd.collective_compute(
                        "AllGather", mybir.AluOpType.bypass,
                        replica_groups=CC_PAIRS,
                        ins=[cc_k_in[:, :, :]], outs=[cc_k_out[:, :, :, :]])
                    for kb in range(KB // 2):
                        nc.sync.dma_start(cc_v_in[:, kb, :], v_sb[:, kb, :])
                    nc.gpsimd.collective_compute(
                        "AllGather", mybir.AluOpType.bypass,
                        replica_groups=CC_PAIRS,
                        ins=[cc_v_in[:, :, :]], outs=[cc_v_out[:, :, :, :]])
                    # re-load in global order (both halves, both cores)
                    for hf in range(2):
                        for ci in range(CI):
                            nc.scalar.dma_start(
                                kt_sb[:, ci, hf * (T // 2):(hf + 1) * (T // 2)],
                                cc_k_out[hf, :, ci, :])
                        for kb in range(KB // 2):
                            nc.scalar.dma_start(
                                v_sb[:, hf * (KB // 2) + kb, :],
                                cc_v_out[hf, :, kb, :])

                # Q^T [cout, rows] -> qt_sb
                for qch in range(TOWN // 512):
                    xq_sb = xa.tile([128, CI, 512], BF16, tag="xa")
                    nc.sync.dma_start(
                        xq_sb[:], xtq3[:, :, qch * 512:(qch + 1) * 512])
                    for co in range(CO):
                        qps = ps_q.tile([128, 512], F32, tag="qps")
                        for ci in range(CI):
                            nc.tensor.matmul(
                                qps[:], wq_sb[:, ci, co * 128:(co + 1) * 128],
                                xq_sb[:, ci, :], start=(ci == 0), stop=(ci == CI - 1))
                        nc.vector.tensor_copy(
                            qt_sb[:, co, qch * 512:(qch + 1) * 512], qps[:])

            # wo on the gpsimd (SWDGE) queue: latency-insensitive
            for ci in range(CI):
                nc.gpsimd.dma_start(wo_sb[:, ci, :], w3["wot"][:, ci, :])

            # ================= Phase B: attention + out-proj ================
            with tc.tile_pool(name="et", bufs=1) as etp, \
                 tc.tile_pool(name="wb", bufs=3) as wb, \
                 tc.tile_pool(name="ysb", bufs=2) as ysb_p, \
                 tc.tile_pool(name="zst", bufs=3) as zstp, \
                 tc.tile_pool(name="ps_s", bufs=3, space="PSUM") as ps_s, \
                 tc.tile_pool(name="ps_sh", bufs=1, space="PSUM") as ps_sh, \
                 tc.tile_pool(name="ps_y", bufs=2, space="PSUM") as ps_y, \
                 tc.tile_pool(name="ps_z", bufs=2, space="PSUM") as ps_z:
                et = etp.tile([128, ETOT], BF16, tag="et")

                def do_slot(s):
                    nkb = CAPS[s]
                    kbs = list(range(KB - 1, KB - 1 - nkb, -1))
                    ecol = {kb: EOFF[kb] + RCH * s for kb in kbs}
                    # key-sums via ones matmul (E is zero at masked positions)
                    sums_ps = ps_sh.tile([1, RCH], F32, tag="shared")
                    for i, kb in enumerate(kbs):
                        nc.tensor.matmul(
                            sums_ps[:], ones_bf[:], et[:, ecol[kb]:ecol[kb] + RCH],
                            start=(i == 0), stop=(i == nkb - 1))
                    recip = wb.tile([1, RCH], F32R, tag="recip")
                    with nc.allow_low_precision(reason="fp32r normalizer"):
                        nc.vector.reciprocal(recip[:], sums_ps[:])
                    rb_ps = ps_sh.tile([128, RCH], F32, tag="shared")
                    nc.tensor.matmul(rb_ps[:], ones_1x128[:], recip[:],
                                     start=True, stop=True)
                    rb_sb = wb.tile([128, RCH], F32, tag="rbsb")
                    nc.vector.tensor_copy(rb_sb[:], rb_ps[:])
                    # Y^T = V @ E^T per cout block
                    y_sb = ysb_p.tile([128, CO, RCH], BF16, tag="ysb")
                    for co in range(CO):
                        yps = ps_y.tile([128, RCH], F32, tag="yps")
                        for i, kb in enumerate(kbs):
                            nc.tensor.matmul(
                                yps[:], v_sb[:, kb, co * 128:(co + 1) * 128],
                                et[:, ecol[kb]:ecol[kb] + RCH],
                                start=(i == 0), stop=(i == nkb - 1))
                        nc.vector.tensor_copy(y_sb[:, co, :], yps[:])
                    # out-proj + normalize
                    for co in range(CO):
                        zps = ps_z.tile([128, RCH], F32, tag="zps")
                        for ci in range(CI):
                            nc.tensor.matmul(
                                zps[:], wo_sb[:, ci, co * 128:(co + 1) * 128],
                                y_sb[:, ci, :], start=(ci == 0), stop=(ci == CI - 1))
                        zst = zstp.tile([128, RCH], F32, tag="zst")
                        nc.vector.tensor_tensor(zst[:], zps[:], rb_sb[:],
                                                mybir.AluOpType.mult)
                        nc.sync.dma_start(
                            zt[co * 128:(co + 1) * 128, s * RCH:(s + 1) * RCH],
                            zst[:])

                # kb-major S sweep; finalize slot s right after its last kb
                for kb in range(KB - 1, -1, -1):
                    w = SPAN[kb] * RCH
                    e0 = EOFF[kb]
                    for c0 in range(0, w, 512):
                        c1 = min(c0 + 512, w)
                        sps = ps_s.tile([128, c1 - c0], F32, tag="sps")
                        for ci in range(CI):
                            nc.tensor.matmul(
                                sps[:], kt_sb[:, ci, kb * 128:(kb + 1) * 128],
                                qt_sb[:, ci, c0:c1],
                                start=(ci == 0), stop=(ci == CI - 1))
                        nc.scalar.activation(et[:, e0 + c0:e0 + c1], sps[:],
                                             mybir.ActivationFunctionType.Exp,
                                             scale=inv_sqrt_c)
                    m = NMASK[kb]
                    if m:
                        mc0 = (SPAN[kb] - m) * RCH  # col offset of masked slots
                        mask = wb.tile([128, m * RCH], BF16, tag="mask")
                        nc.vector.tensor_tensor(
                            mask[:],
                            ki16_sb[:, kb:kb + 1].to_broadcast((128, m * RCH)),
                            ri_sb[:, mc0:mc0 + m * RCH], mybir.AluOpType.is_ge)
                        nc.vector.tensor_tensor(
                            et[:, e0 + mc0:e0 + mc0 + m * RCH],
                            et[:, e0 + mc0:e0 + mc0 + m * RCH], mask[:],
                            mybir.AluOpType.mult)
                    for s in range(NSLOT):
                        if KB - CAPS[s] == kb:  # slot s complete at this kb
                            do_slot(s)
    nc.compile()
    _NC_CACHE["nc"] = nc
    return nc


def make_in_maps(inputs):
    x = np.asarray(inputs["x"], dtype=np.float32)
    for bname in ("bq", "bk", "bv", "bo"):
        bval = np.asarray(inputs[bname])
        assert np.all(bval == 0.0), f"{bname} nonzero: unsupported fast path"
    bf = ml_dtypes.bfloat16
    wqt = np.ascontiguousarray(np.asarray(inputs["Wq"], np.float32).T.astype(bf))
    wkt = np.ascontiguousarray(np.asarray(inputs["Wk"], np.float32).T.astype(bf))
    wvt = np.ascontiguousarray(np.asarray(inputs["Wv"], np.float32).T.astype(bf))
    wot = np.ascontiguousarray(np.asarray(inputs["Wo"], np.float32).T.astype(bf))
    keyidx16 = (np.arange(T, dtype=np.float32).reshape(KB, 128).T + WINDOW
                ).copy()  # [128, KB]
    in_maps = []
    for core in range(N_CORES):
        b, h = divmod(core, 2)
        xt_b = np.ascontiguousarray(x[b].T.astype(bf))  # [C, T]
        chunks = CHUNK_MAP[h]
        xtq = np.concatenate(
            [xt_b[:, c * RCH:(c + 1) * RCH] for c in chunks], axis=1)
        rowidx = np.concatenate(
            [np.arange(c * RCH, (c + 1) * RCH, dtype=np.float32)
             for c in chunks])
        rowidxb = np.ascontiguousarray(
            np.broadcast_to(rowidx[None, :], (128, TOWN)))
        xt_in = (np.ascontiguousarray(xt_b[:, h * (T // 2):(h + 1) * (T // 2)])
                 if USE_CC else xt_b)
        in_maps.append({
            "xt": xt_in, "xtq": np.ascontiguousarray(xtq),
            "wqt": wqt, "wkt": wkt, "wvt": wvt, "wot": wot,
            "keyidx16": keyidx16, "rowidxb": rowidxb,
        })
    return in_maps


def gather_output(results, dtype):
    out = np.empty((B, T, C), dtype=dtype)
    for core in range(N_CORES):
        b, h = divmod(core, 2)
        y = results[core]["zt"].T  # [TOWN rows, C]
        for si, c in enumerate(CHUNK_MAP[h]):
            out[b, c * RCH:(c + 1) * RCH] = y[si * RCH:(si + 1) * RCH]
    return out


def kernel(**inputs):
    nc = build()
    in_maps = make_in_maps(inputs)
    res = bass_utils.run_bass_kernel_spmd(nc, in_maps,
                                          core_ids=list(range(N_CORES)))
    return gather_output(res.results, np.asarray(inputs["x"]).dtype)


# revision 11
# speedup vs baseline: 1.4024x; 1.1144x over previous
"""TRN2 Bass kernel v2 for nn_LocalAttention (B=4, T=2048, C=1024, window=16).

Sharding: 8 cores = (batch b, half h). Each core handles 1024 rows of one
batch as four 256-row slots; h=0 gets global 256-chunks (0,3,4,7), h=1 gets
(1,2,5,6) (balanced by kept-key-block count, slot needs differ by <=2 from
the uniform caps (16,13,9,5)).

All matmul operands are bf16 (host-cast; PSUM accumulates fp32). K^T, V, Q^T
are SBUF-resident - no DRAM round-trip for attention operands.

Mask keeps j >= i - 16 (reverse-causal): each slot's kept key-block set is a
suffix; processing key blocks in DESCENDING order makes every kept set a
static prefix. The S sweep is kb-major: for each key block, one wide matmul
over all slots still needing it (slot spans are prefixes of the row axis
because slot caps are descending). E is packed [128 keys, sum(span)] bf16.
Only trailing span slots can touch the diagonal/boundary, so just those get
the data-driven is_ge mask (zeroes overflow positions too).

Optional USE_CC: each core projects K^T only for its own half of T (from the
extra xth input) and a pair-wise (cores 2b,2b+1) DRAM AllGather assembles the
full K^T, halving the K projection work. V and Q stay fully local; the ~65us
gather hides under the V+Q projections. Staging reuses kt_sb's first half;
the gather output overwrites both halves in global key order on both cores.
"""
import numpy as np
import ml_dtypes

import concourse.bass as bass
import concourse.mybir as mybir
import concourse.tile as tile
from concourse import bacc
from concourse import bass_utils

N_CORES = 8
B, T, C = 4, 2048, 1024
WINDOW = 16
TOWN = T // 2           # own rows per core
RCH = 256               # rows per slot
NSLOT = TOWN // RCH     # 4
CI = C // 128           # 8 contraction blocks
CO = C // 128           # 8 output blocks
KB = T // 128           # 16 key blocks
CAPS = (16, 13, 9, 5)   # key-block positions per slot (descending kb)
CHUNK_MAP = {0: (0, 3, 4, 7), 1: (1, 2, 5, 6)}  # slot -> global 256-chunk

F32 = mybir.dt.float32
F32R = mybir.dt.float32r
BF16 = mybir.dt.bfloat16

USE_CC = True
CC_PAIRS = [[0, 1], [2, 3], [4, 5], [6, 7]]

# ---- static kb-major S-sweep structure ----
# span n(kb): number of slots (prefix of slot axis) needing key block kb
SPAN = {kb: sum(1 for cap in CAPS if cap >= KB - kb) for kb in range(KB)}
EOFF = {}
_off = 0
for _kb in range(KB - 1, -1, -1):
    EOFF[_kb] = _off
    _off += SPAN[_kb] * RCH
ETOT = _off  # 11008

# masked trailing slots per kb: slot s is mask-free at kb iff kb is strictly
# above the diagonal for BOTH cores' chunk at that slot
_MAXCHUNK = [max(CHUNK_MAP[0][s], CHUNK_MAP[1][s]) for s in range(NSLOT)]
NMASK = {}
for _kb in range(KB):
    n = SPAN[_kb]
    safe = [_kb >= 2 * _MAXCHUNK[s] + 2 for s in range(n)]
    # safe flags must be a prefix (maxchunk increases with s)
    assert safe == sorted(safe, reverse=True), (_kb, safe)
    NMASK[_kb] = n - sum(safe)

inv_sqrt_c = float(1.0 / np.sqrt(C))

_NC_CACHE = {}


def build():
    if "nc" in _NC_CACHE:
        return _NC_CACHE["nc"]
    nc = bacc.Bacc("TRN2", target_bir_lowering=False, debug=False,
                   num_devices=N_CORES)
    xt = nc.dram_tensor("xt", [C, T], BF16, kind="ExternalInput").ap()
    if USE_CC:
        xth = nc.dram_tensor("xth", [C, T // 2], BF16, kind="ExternalInput").ap()
        xth3 = xth.rearrange("(ko ki) t -> ki ko t", ki=128)
    xtq = nc.dram_tensor("xtq", [C, TOWN], BF16, kind="ExternalInput").ap()
    wqt = nc.dram_tensor("wqt", [C, C], BF16, kind="ExternalInput").ap()
    wkt = nc.dram_tensor("wkt", [C, C], BF16, kind="ExternalInput").ap()
    wvt = nc.dram_tensor("wvt", [C, C], BF16, kind="ExternalInput").ap()
    wot = nc.dram_tensor("wot", [C, C], BF16, kind="ExternalInput").ap()
    keyidx16 = nc.dram_tensor("keyidx16", [128, KB], F32, kind="ExternalInput").ap()
    rowidxb = nc.dram_tensor("rowidxb", [128, TOWN], F32, kind="ExternalInput").ap()
    zt = nc.dram_tensor("zt", [C, TOWN], F32, kind="ExternalOutput").ap()

    xt3 = xt.rearrange("(ko ki) t -> ki ko t", ki=128)
    xtq3 = xtq.rearrange("(ko ki) t -> ki ko t", ki=128)
    w3 = {w.tensor.name: w.rearrange("(ko ki) c -> ki ko c", ki=128)
          for w in (wqt, wkt, wvt, wot)}

    if USE_CC:
        cc_k_in = nc.dram_tensor("cc_k_in", [128, CI, T // 2], BF16).ap()
        cc_k_out = nc.dram_tensor("cc_k_out", [2, 128, CI, T // 2], BF16).ap()

    with tile.TileContext(nc) as tc:
        with tc.tile_pool(name="res", bufs=1) as res:
            kt_sb = res.tile([128, CI, T], BF16, tag="kt")    # K^T resident
            v_sb = res.tile([128, KB, C], BF16, tag="v")      # V resident
            qt_sb = res.tile([128, CI, TOWN], BF16, tag="qt")  # Q^T resident
            wo_sb = res.tile([128, CI, C], BF16, tag="wo")
            ki16_sb = res.tile([128, KB], F32, tag="ki16")
            ri_sb = res.tile([128, TOWN], F32, tag="ri")
            nc.gpsimd.dma_start(ki16_sb[:], keyidx16[:])
            nc.gpsimd.dma_start(ri_sb[:], rowidxb[:])
            ones_col_f32 = res.tile([128, 1], F32, tag="onescf")
            nc.vector.memset(ones_col_f32[:], 1.0)
            ones_bf = res.tile([128, 1], BF16, tag="o1")
            nc.vector.tensor_copy(ones_bf[:], ones_col_f32[:])
            ones_row_f32 = res.tile([1, 128], F32, tag="onesrf")
            nc.vector.memset(ones_row_f32[:], 1.0)
            ones_1x128 = res.tile([1, 128], F32R, tag="o2")
            nc.vector.tensor_copy(ones_1x128[:], ones_row_f32[:])

            # ================= Phase A: projections =========================
            with tc.tile_pool(name="wts", bufs=1) as wts, \
                 tc.tile_pool(name="xa", bufs=2) as xa, \
                 tc.tile_pool(name="ps_k", bufs=3, space="PSUM") as ps_k, \
                 tc.tile_pool(name="ps_v", bufs=2, space="PSUM") as ps_v, \
                 tc.tile_pool(name="ps_q", bufs=2, space="PSUM") as ps_q:
                wk_sb = wts.tile([128, CI, C], BF16, tag="wk")
                wv_sb = wts.tile([128, CI, C], BF16, tag="wv")
                wq_sb = wts.tile([128, CI, C], BF16, tag="wq")
                # first x chunk on the sync queue ahead of everything else;
                # wk per-column on scalar so x and wk stream in parallel
                k_src3 = xth3 if USE_CC else xt3
                n_ktch = (T // 2 if USE_CC else T) // 512
                xk_sb0 = xa.tile([128, CI, 512], BF16, tag="xa")
                nc.sync.dma_start(xk_sb0[:],
                                  k_src3[:, :, (n_ktch - 1) * 512:n_ktch * 512])
                for co in range(CO):  # per-column loads: co=0 unblocks MMs
                    nc.scalar.dma_start(wk_sb[:, :, co * 128:(co + 1) * 128],
                                        w3["wkt"][:, :, co * 128:(co + 1) * 128])
                for ci in range(CI):
                    nc.scalar.dma_start(wv_sb[:, ci, :], w3["wvt"][:, ci, :])
                for ci in range(CI):
                    nc.scalar.dma_start(wq_sb[:, ci, :], w3["wqt"][:, ci, :])

                def k_chunk(xt_sb, tch):
                    for co in range(CO):
                        kps = ps_k.tile([128, 512], F32, tag="kps")
                        for ci in range(CI):
                            nc.tensor.matmul(
                                kps[:], wk_sb[:, ci, co * 128:(co + 1) * 128],
                                xt_sb[:, ci, :], start=(ci == 0), stop=(ci == CI - 1))
                        nc.vector.tensor_copy(
                            kt_sb[:, co, tch * 512:(tch + 1) * 512], kps[:])

                def v_chunk(xt_sb, tch):
                    for tb in range(4):
                        for half in range(2):
                            vps = ps_v.tile([128, 512], F32, tag="vps")
                            for ci in range(CI):
                                nc.tensor.matmul(
                                    vps[:], xt_sb[:, ci, tb * 128:(tb + 1) * 128],
                                    wv_sb[:, ci, half * 512:(half + 1) * 512],
                                    start=(ci == 0), stop=(ci == CI - 1))
                            nc.vector.tensor_copy(
                                v_sb[:, tch * 4 + tb,
                                     half * 512:(half + 1) * 512], vps[:])

                if USE_CC:
                    # K^T for own half only, then pair AllGather of the halves
                    for tch in reversed(range(n_ktch)):
                        if tch == n_ktch - 1:
                            xk_sb = xk_sb0
                        else:
                            xk_sb = xa.tile([128, CI, 512], BF16, tag="xa")
                            nc.sync.dma_start(
                                xk_sb[:], xth3[:, :, tch * 512:(tch + 1) * 512])
                        k_chunk(xk_sb, tch)
                    for ci in range(CI):
                        nc.sync.dma_start(cc_k_in[:, ci, :],
                                          kt_sb[:, ci, 0:T // 2])
                    nc.gpsimd.collective_compute(
                        "AllGather", mybir.AluOpType.bypass,
                        replica_groups=CC_PAIRS,
                        ins=[cc_k_in[:, :, :]], outs=[cc_k_out[:, :, :, :]])
                    # V over full T while the gather is in flight
                    for tch in reversed(range(T // 512)):
                        xt_sb = xa.tile([128, CI, 512], BF16, tag="xa")
                        nc.sync.dma_start(
                            xt_sb[:], xt3[:, :, tch * 512:(tch + 1) * 512])
                        v_chunk(xt_sb, tch)
                else:
                    for tch in reversed(range(T // 512)):
                        if tch == n_ktch - 1:
                            xt_sb = xk_sb0
                        else:
                            xt_sb = xa.tile([128, CI, 512], BF16, tag="xa")
                            nc.sync.dma_start(
                                xt_sb[:], xt3[:, :, tch * 512:(tch + 1) * 512])
                        k_chunk(xt_sb, tch)
                        v_chunk(xt_sb, tch)

                # Q^T [cout, rows] -> qt_sb
                for qch in range(TOWN // 512):
                    xq_sb = xa.tile([128, CI, 512], BF16, tag="xa")
                    nc.sync.dma_start(
                        xq_sb[:], xtq3[:, :, qch * 512:(qch + 1) * 512])
                    for co in range(CO):
                        qps = ps_q.tile([128, 512], F32, tag="qps")
                        for ci in range(CI):
                            nc.tensor.matmul(
                                qps[:], wq_sb[:, ci, co * 128:(co + 1) * 128],
                                xq_sb[:, ci, :], start=(ci == 0), stop=(ci == CI - 1))
                        nc.vector.tensor_copy(
                            qt_sb[:, co, qch * 512:(qch + 1) * 512], qps[:])

                if USE_CC:
                    # re-load full K^T in global key order (high half first:
                    # the S sweep consumes key blocks in descending order)
                    for hf in (1, 0):
                        for ci in range(CI):
                            nc.scalar.dma_start(
                                kt_sb[:, ci, hf * (T // 2):(hf + 1) * (T // 2)],
                                cc_k_out[hf, :, ci, :])

            # wo on the gpsimd (SWDGE) queue: latency-insensitive
            for ci in range(CI):
                nc.gpsimd.dma_start(wo_sb[:, ci, :], w3["wot"][:, ci, :])

            # ================= Phase B: attention + out-proj ================
            with tc.tile_pool(name="et", bufs=1) as etp, \
                 tc.tile_pool(name="wb", bufs=3) as wb, \
                 tc.tile_pool(name="ysb", bufs=2) as ysb_p, \
                 tc.tile_pool(name="zst", bufs=3) as zstp, \
                 tc.tile_pool(name="ps_s", bufs=3, space="PSUM") as ps_s, \
                 tc.tile_pool(name="ps_sh", bufs=1, space="PSUM") as ps_sh, \
                 tc.tile_pool(name="ps_y", bufs=2, space="PSUM") as ps_y, \
                 tc.tile_pool(name="ps_z", bufs=2, space="PSUM") as ps_z:
                et = etp.tile([128, ETOT], BF16, tag="et")

                def do_slot(s):
                    nkb = CAPS[s]
                    kbs = list(range(KB - 1, KB - 1 - nkb, -1))
                    ecol = {kb: EOFF[kb] + RCH * s for kb in kbs}
                    # key-sums via ones matmul (E is zero at masked positions)
                    sums_ps = ps_sh.tile([1, RCH], F32, tag="shared")
                    for i, kb in enumerate(kbs):
                        nc.tensor.matmul(
                            sums_ps[:], ones_bf[:], et[:, ecol[kb]:ecol[kb] + RCH],
                            start=(i == 0), stop=(i == nkb - 1))
                    recip = wb.tile([1, RCH], F32R, tag="recip")
                    with nc.allow_low_precision(reason="fp32r normalizer"):
                        nc.vector.reciprocal(recip[:], sums_ps[:])
                    rb_ps = ps_sh.tile([128, RCH], F32, tag="shared")
                    nc.tensor.matmul(rb_ps[:], ones_1x128[:], recip[:],
                                     start=True, stop=True)
                    rb_sb = wb.tile([128, RCH], F32, tag="rbsb")
                    nc.vector.tensor_copy(rb_sb[:], rb_ps[:])
                    # Y^T = V @ E^T per cout block
                    y_sb = ysb_p.tile([128, CO, RCH], BF16, tag="ysb")
                    for co in range(CO):
                        yps = ps_y.tile([128, RCH], F32, tag="yps")
                        for i, kb in enumerate(kbs):
                            nc.tensor.matmul(
                                yps[:], v_sb[:, kb, co * 128:(co + 1) * 128],
                                et[:, ecol[kb]:ecol[kb] + RCH],
                                start=(i == 0), stop=(i == nkb - 1))
                        nc.vector.tensor_copy(y_sb[:, co, :], yps[:])
                    # out-proj + normalize
                    for co in range(CO):
                        zps = ps_z.tile([128, RCH], F32, tag="zps")
                        for ci in range(CI):
                            nc.tensor.matmul(
                                zps[:], wo_sb[:, ci, co * 128:(co + 1) * 128],
                                y_sb[:, ci, :], start=(ci == 0), stop=(ci == CI - 1))
                        zst = zstp.tile([128, RCH], F32, tag="zst")
                        nc.vector.tensor_tensor(zst[:], zps[:], rb_sb[:],
                                                mybir.AluOpType.mult)
                        nc.sync.dma_start(
                            zt[co * 128:(co + 1) * 128, s * RCH:(s + 1) * RCH],
                            zst[:])

                # kb-major S sweep; finalize slot s right after its last kb
                for kb in range(KB - 1, -1, -1):
                    w = SPAN[kb] * RCH
                    e0 = EOFF[kb]
                    for c0 in range(0, w, 512):
                        c1 = min(c0 + 512, w)
                        sps = ps_s.tile([128, c1 - c0], F32, tag="sps")
                        for ci in range(CI):
                            nc.tensor.matmul(
                                sps[:], kt_sb[:, ci, kb * 128:(kb + 1) * 128],
                                qt_sb[:, ci, c0:c1],
                                start=(ci == 0), stop=(ci == CI - 1))
                        nc.scalar.activation(et[:, e0 + c0:e0 + c1], sps[:],
                                             mybir.ActivationFunctionType.Exp,
                                             scale=inv_sqrt_c)
                    m = NMASK[kb]
                    if m:
                        mc0 = (SPAN[kb] - m) * RCH  # col offset of masked slots
                        mask = wb.tile([128, m * RCH], BF16, tag="mask")
                        nc.vector.tensor_tensor(
                            mask[:],
                            ki16_sb[:, kb:kb + 1].to_broadcast((128, m * RCH)),
                            ri_sb[:, mc0:mc0 + m * RCH], mybir.AluOpType.is_ge)
                        nc.vector.tensor_tensor(
                            et[:, e0 + mc0:e0 + mc0 + m * RCH],
                            et[:, e0 + mc0:e0 + mc0 + m * RCH], mask[:],
                            mybir.AluOpType.mult)
                    for s in range(NSLOT):
                        if KB - CAPS[s] == kb:  # slot s complete at this kb
                            do_slot(s)
    nc.compile()
    _NC_CACHE["nc"] = nc
    return nc


def make_in_maps(inputs):
    x = np.asarray(inputs["x"], dtype=np.float32)
    for bname in ("bq", "bk", "bv", "bo"):
        bval = np.asarray(inputs[bname])
        assert np.all(bval == 0.0), f"{bname} nonzero: unsupported fast path"
    bf = ml_dtypes.bfloat16
    wqt = np.ascontiguousarray(np.asarray(inputs["Wq"], np.float32).T.astype(bf))
    wkt = np.ascontiguousarray(np.asarray(inputs["Wk"], np.float32).T.astype(bf))
    wvt = np.ascontiguousarray(np.asarray(inputs["Wv"], np.float32).T.astype(bf))
    wot = np.ascontiguousarray(np.asarray(inputs["Wo"], np.float32).T.astype(bf))
    keyidx16 = (np.arange(T, dtype=np.float32).reshape(KB, 128).T + WINDOW
                ).copy()  # [128, KB]
    in_maps = []
    for core in range(N_CORES):
        b, h = divmod(core, 2)
        xt_b = np.ascontiguousarray(x[b].T.astype(bf))  # [C, T]
        chunks = CHUNK_MAP[h]
        xtq = np.concatenate(
            [xt_b[:, c * RCH:(c + 1) * RCH] for c in chunks], axis=1)
        rowidx = np.concatenate(
            [np.arange(c * RCH, (c + 1) * RCH, dtype=np.float32)
             for c in chunks])
        rowidxb = np.ascontiguousarray(
            np.broadcast_to(rowidx[None, :], (128, TOWN)))
        im = {
            "xt": xt_b, "xtq": np.ascontiguousarray(xtq),
            "wqt": wqt, "wkt": wkt, "wvt": wvt, "wot": wot,
            "keyidx16": keyidx16, "rowidxb": rowidxb,
        }
        if USE_CC:
            im["xth"] = np.ascontiguousarray(
                xt_b[:, h * (T // 2):(h + 1) * (T // 2)])
        in_maps.append(im)
    return in_maps


def gather_output(results, dtype):
    out = np.empty((B, T, C), dtype=dtype)
    for core in range(N_CORES):
        b, h = divmod(core, 2)
        y = results[core]["zt"].T  # [TOWN rows, C]
        for si, c in enumerate(CHUNK_MAP[h]):
            out[b, c * RCH:(c + 1) * RCH] = y[si * RCH:(si + 1) * RCH]
    return out


def kernel(**inputs):
    nc = build()
    in_maps = make_in_maps(inputs)
    res = bass_utils.run_bass_kernel_spmd(nc, in_maps,
                                          core_ids=list(range(N_CORES)))
    return gather_output(res.results, np.asarray(inputs["x"]).dtype)
